# revision 25
# baseline (speedup 1.0000x reference)
"""Multi-head attention (S=2048, B=2, D=1024, H=16) on 8 Trainium2 NeuronCores.

Sharding: batch x head-group. Core c handles batch c//4 and heads
[4*(c%4), 4*(c%4)+4). Each core computes its 4 heads' Q/K/V projections,
attention, and a partial output projection (row-parallel Wo); the host sums
the 4 partials per batch and adds the bias terms (bo and the exact wo@bv
correction; softmax rows sum to 1 so bv folds out of the attention).

v4 device-side structure (per core):
  - x inputs and all weights arrive as bf16 (host-converted), halving HBM
    traffic; projections run bf16 through the PE at 1 cycle/row.
  - Q is projected PRE-SCALED by K' = SCALE*1024/ln2, so score PSUM tiles
    hold T = s*SCALE*1024/ln2 directly (the fp16 exponent-unit domain).
  - exp splits across ACT and DVE per tick parity:
      ACT: activation(Exp, scale=ln2/1024) -> exact fp16.
      DVE: one fused custom op (EXP2_FUSED_ANT, 8 ALU stages): magic-number
      floor-to-1024 grid, quadratic mantissa correction
      mant = F + c*F*(F-1024), c = 0.34/1024 -> int16 bit pattern that IS
      fp16(exp(s*SCALE)) with ~0.3% max error (vs ~2% Schraudolph).
  - All other elementwise work (bias adds, conversions, 1/Z normalize,
    stage copies) is greedily load-balanced across ACT, DVE, and Pool
    (GPSIMD), keeping ACT/DVE exp throughput ahead of the PE.
  - P@V runs fp16 with a ones-column appended to V so the PE accumulates
    the softmax denominator Z next to U; normalization is deferred past
    P@V (one reciprocal + per-partition-scalar multiplies).
  - PSUM is partitioned to decouple the pipeline: score tiles get a
    dedicated 2-deep rotation (4 banks), the two live U accumulators get
    their own banks, and projection/transpose/output-projection scratch
    rotates in a third pool, so scores(t+1) never serialize behind exp(t).
  - P@V is emitted four ticks behind scores/exp so its matmuls never park
    in the PE wait queue (depth 4) blocking later scores; input slabs land
    as two half-DMAs so projections start after half the transfer.
  - O = U * (1/Z) -> bf16, PE-transposed, output projection in bf16;
    y partials fly to HBM as bf16 and the host sums them in f32.
"""

import sys

sys.path.insert(0, "/opt/trn_rl_repo")

from collections import deque

import numpy as np
import ml_dtypes

import concourse.bass as bass
import concourse.tile as tile
from concourse import bacc, mybir
from concourse.bass_utils import run_bass_kernel_spmd
from concourse.masks import make_identity


S = 2048
B = 2
D = 1024
H = 16
DK = 64
G = 4            # heads per core
DC = G * DK      # 256 per-core head dims
SCALE = 1.0 / np.sqrt(DK)  # 0.125
P = 128
NSC = 4          # 512-col s-chunks (query i-blocks and proj chunks)
SC = S // NSC    # 512
NJ = 16          # 128-row j chunks
ND = D // P      # 8 contraction chunks for projections

F32 = mybir.dt.float32
F32R = mybir.dt.float32r
BF16 = mybir.dt.bfloat16
I16 = mybir.dt.int16
FP16 = mybir.dt.float16
EXP = mybir.ActivationFunctionType.Exp
IDENT = mybir.ActivationFunctionType.Identity
COPY = mybir.ActivationFunctionType.Copy
ADD = mybir.AluOpType.add
MULT = mybir.AluOpType.mult

# Q pre-scale: score PSUM holds T = s * SCALE * 1024 / ln2
KPRE = float(SCALE * 1024.0 / np.log(2.0))
LN2_1024 = float(np.log(2.0) / 1024.0)
EXPC = 0.34 / 1024.0         # quadratic mantissa-correction constant
EXP_BIAS = 15360.0           # fp16 exponent bias in mantissa counts (15<<10)
EXP_MAGIC = float(2.0**33 - 512.0)  # f32 round-to-1024-grid magic (exact)
EXPC_SHIFT = float(-EXPC * 262144.0)  # -c*512^2, via the C3/Src1 latch

_NC_CACHE = None
_EXP2_OP = None


def _get_exp2_op():
    """Fused DVE op: in0 = T (f32); out int16 = fp16 bit pattern of
    2^(T/1024) with quadratic mantissa correction.

      T3 = T + B;  r = (T3+C1)-C1, C1 = M-512   [floor-to-1024 grid]
      Ft = T3 - r = F - 512  in [-512, 512)
      y  = (T3 - c*512^2) + c*Ft^2   [= T + B + c*F*(F-1024)] -> round -> u16

    Scalars: s0 = B (=15360), s1 = M-512, imm2 = c; in1 = [P,1] tile
    holding -c*512^2 (delivered via the C3/Src1 latch).
    """
    global _EXP2_OP
    if _EXP2_OP is not None:
        return _EXP2_OP
    from concourse import dve_ops
    from concourse.dve_spec import Spec, Src0, C0, C1, C2, C3, lower
    from concourse.dve_ops import DveOp, _spill_c3_to_src1, has_src1
    from concourse.dve_uop import DveOpSpec

    name = "EXP2_FUSED_ANT"
    T3 = Src0 + C0
    u = T3 + C1
    r = u - C1
    Ft = T3 - r
    Q2 = Ft * Ft
    Pn = Q2 * C2
    T3b = T3 + C3
    body = _spill_c3_to_src1(T3b + Pn)

    def _ref(in0, in1, s0, s1, imm2):
        f = np.float32
        cq = np.asarray(in1, f).reshape(in1.shape[0], -1)[:, :1]
        T3 = (np.asarray(in0, f) + f(s0)).astype(f)
        u = (T3 + f(s1)).astype(f)
        r = (u - f(s1)).astype(f)
        Ft = (T3 - r).astype(f)
        Q2 = (Ft * Ft).astype(f)
        Pn = (Q2 * f(imm2)).astype(f)
        T3b = (T3 + cq).astype(f)
        y = (T3b + Pn).astype(f)
        return np.rint(y).astype(f)

    spec = Spec(body=body, reference=_ref)
    row = 17
    dve_ops._SUB_OPCODE_FOR_NAME[name] = row
    shas = {}
    for ver in ("v3", "v4"):
        try:
            s = DveOpSpec(
                name=name, opcode=row, uops=lower(spec, ver=ver),
                rd1_en=has_src1(spec),
            )
            shas[ver] = s.sha(ver)
        except Exception:
            pass
    op = DveOp(name, spec, subdim=False, uops_sha=shas)
    dve_ops.OPS.append(op)
    dve_ops.CUSTOM_DVE_SPECS[name] = spec
    _EXP2_OP = op
    return op


def _build():
    nc = bacc.Bacc("TRN2", target_bir_lowering=False, debug=False)
    exp2_op = _get_exp2_op()

    xq_t = nc.dram_tensor("xq_t", [D, S], BF16, kind="ExternalInput")
    xk_t = nc.dram_tensor("xk_t", [D, S], BF16, kind="ExternalInput")
    xv_t = nc.dram_tensor("xv_t", [D, S], BF16, kind="ExternalInput")
    wq_t = nc.dram_tensor("wq_t", [D, DC], BF16, kind="ExternalInput")
    wk_t = nc.dram_tensor("wk_t", [D, DC], BF16, kind="ExternalInput")
    wv_t = nc.dram_tensor("wv_t", [D, DC], BF16, kind="ExternalInput")
    wo_t = nc.dram_tensor("wo_t", [DC, D], BF16, kind="ExternalInput")
    bq_s = nc.dram_tensor("bq_s", [P, 2], F32, kind="ExternalInput")  # pre-scaled by KPRE
    bk_s = nc.dram_tensor("bk_s", [P, 2], F32, kind="ExternalInput")
    y = nc.dram_tensor("y", [S, D], BF16, kind="ExternalOutput")

    # greedy two-engine load balancer for elementwise work (Pool/GPSIMD
    # cannot access PSUM, and every elementwise op here reads PSUM).
    ew = {"A": 0.0, "D": 0.0}

    def pick(cost_a, cost_d, force=None):
        if force is None:
            e = "A" if ew["A"] + cost_a <= ew["D"] + cost_d else "D"
        else:
            e = force
        ew[e] += cost_a if e == "A" else cost_d
        return e

    def c_act(n):  # ACT engine busy ns for n-col op
        return n * 0.833 + 185
    def c_dve(n, half=False):
        return n * (0.521 if half else 1.042) + 125

    with tile.TileContext(nc) as tc:
        with (
            tc.tile_pool(name="persist", bufs=1) as persist,
            tc.tile_pool(name="xs", bufs=8) as xs,
            tc.tile_pool(name="stp", bufs=2, space="PSUM") as stp,
            tc.tile_pool(name="ub", bufs=2, space="PSUM") as ub,
            tc.tile_pool(name="scr", bufs=2, space="PSUM") as scr,
            tc.tile_pool(name="et", bufs=12) as etp,
            tc.tile_pool(name="rz", bufs=4) as rzp,
            tc.tile_pool(name="ysb", bufs=6) as ysb,
        ):
            # ---- persistent SBUF ----
            wq_sb = persist.tile([P, ND, DC], BF16)
            wk_sb = persist.tile([P, ND, DC], BF16)
            wv_sb = persist.tile([P, ND, DC], BF16)
            bq_sb = persist.tile([P, 2], F32)
            bk_sb = persist.tile([P, 2], F32)
            expc_sb = persist.tile([P, 1], F32)
            wk_ap = wk_t.ap().rearrange("(c p) m -> p c m", p=P)
            wq_ap = wq_t.ap().rearrange("(c p) m -> p c m", p=P)
            # mt0 column-halves of Wk/Wq land first so the first projection
            # chain (k-mt0-h0 -> q-mt0-h0/h1 -> score tick 0) starts ~5us
            # earlier than with full-weight transfers
            nc.sync.dma_start(out=wk_sb[:, :, 0:P], in_=wk_ap[:, :, 0:P])
            nc.sync.dma_start(out=bk_sb, in_=bk_s.ap())
            nc.sync.dma_start(out=bq_sb, in_=bq_s.ap())
            nc.gpsimd.memset(expc_sb, float(EXPC_SHIFT))
            woc_sb = persist.tile([P, 2, D], BF16)

            qt_sb = [persist.tile([P, S], F32R, name=f"qt{t}") for t in range(2)]
            kt_sb = [persist.tile([P, S], F32R, name=f"kt{t}") for t in range(2)]
            # vaug[p, jc, h, d]; d==64 is the ones column for Z
            vaug = persist.tile([P, NJ, G, 65], FP16)
            nc.vector.memset(vaug[:, :, :, 64:65], 1.0)
            o_sb = persist.tile([P, NSC * 4, DC], BF16)
            ot_sb = [persist.tile([P, S], BF16, name=f"ot{t}") for t in range(2)]
            ident_f = persist.tile([P, P], F32)
            make_identity(nc, ident_f)
            ident = persist.tile([P, P], BF16)
            nc.vector.tensor_copy(ident, ident_f)

            # ---- work items ----
            slabs = {}
            emitted = set()

            def dma_half(key, xt, cb, hf):
                # one 256-col s-half of an input slab: consumers of the first
                # half start after half the transfer latency (subtile deps)
                if (key, cb) not in slabs:
                    slabs[(key, cb)] = xs.tile([P, ND, SC], BF16, tag="x", name="x")
                t = slabs[(key, cb)]
                src_ap = xt.ap().rearrange("(c p) s -> p c s", p=P)
                nc.sync.dma_start(
                    out=t[:, :, hf * HW_ : (hf + 1) * HW_],
                    in_=src_ap[
                        :, :, cb * SC + hf * HW_ : cb * SC + (hf + 1) * HW_
                    ],
                )

            def dma_slab(key, xt, cb):
                for hf in range(2):
                    dma_half(key, xt, cb, hf)
                emitted.add(("dma", key, cb))

            HW_ = SC // 2  # 256-col half for short PSUM holds

            def proj_qk(key, cb, mt, half, w_sb, b_sb, out_tiles, scale):
                ps = scr.tile([P, HW_], F32, tag="scr", name="ps")
                slab = slabs[(key, cb)]
                for dc in range(ND):
                    nc.tensor.matmul(
                        ps,
                        w_sb[:, dc, mt * P : (mt + 1) * P],
                        slab[:, dc, half * HW_ : (half + 1) * HW_],
                        start=(dc == 0),
                        stop=(dc == ND - 1),
                    )
                dst = out_tiles[mt][:, cb * SC + half * HW_ : cb * SC + (half + 1) * HW_]
                e = pick(c_act(HW_), c_dve(HW_))
                if e == "A":
                    nc.scalar.activation(
                        dst, ps, IDENT, bias=b_sb[:, mt : mt + 1], scale=scale
                    )
                else:
                    nc.vector.tensor_scalar(
                        dst, ps, scale, b_sb[:, mt : mt + 1], op0=MULT, op1=ADD
                    )
                emitted.add((key, cb, mt, half))

            def proj_v(jc):
                # j chunk jc (128 rows) -> vaug[:, jc, :, 0:64]
                cb, jq = divmod(jc, 4)
                ps = scr.tile([P, DC], F32, tag="scr", name="ps")
                slab = slabs[("v", cb)]
                for dc in range(ND):
                    nc.tensor.matmul(
                        ps,
                        slab[:, dc, jq * P : (jq + 1) * P],
                        wv_sb[:, dc, :],
                        start=(dc == 0),
                        stop=(dc == ND - 1),
                    )
                dst = vaug[:, jc, :, 0:64]
                src = ps.rearrange("p (h c) -> p h c", h=G)
                e = pick(c_act(DC), c_dve(DC))
                if e == "A":
                    nc.scalar.activation(dst, src, COPY)
                else:
                    nc.vector.tensor_copy(dst, src)
                emitted.add(("v", jc))

            def transp2(ib, mt, pr, fast=False):
                # XBAR DMA transpose (2-byte, SBUF->SBUF): no PE or ACT/DVE
                # work at all; o_sb [i, dc] bf16 -> ot_sb [dc, i].
                # fast=True (kernel tail): PE transpose + A/D copy instead --
                # the PE is idle there and XBAR DMA latency (~1.5us) would
                # sit on the critical path.
                for it in range(ib * 4 + 2 * pr, ib * 4 + 2 * pr + 2):
                    if fast:
                        tp = scr.tile([P, P], BF16, tag="scr", name="tp")
                        nc.tensor.transpose(tp, o_sb[:, it, mt * P : (mt + 1) * P], ident)
                        dst = ot_sb[mt][:, it * P : (it + 1) * P]
                        e = pick(c_act(P), c_dve(P, half=True))
                        if e == "A":
                            nc.scalar.activation(dst, tp, COPY)
                        else:
                            nc.vector.tensor_copy(dst, tp)
                    else:
                        nc.sync.dma_start(
                            out=ot_sb[mt][:, it * P : (it + 1) * P],
                            in_=o_sb[:, it, mt * P : (mt + 1) * P],
                            transpose=True,
                        )

            def oproj(it, fast=False):
                # each half flies to HBM right after its stage copy, and the
                # two copies go to different engines: halves the tail chain
                ys = ysb.tile([P, D], BF16, tag="ysb", name="ysb")
                engines = []
                for nh in range(2):
                    yp = scr.tile([P, SC], F32, tag="scr", name="yp")
                    for mt in range(2):
                        nc.tensor.matmul(
                            yp,
                            ot_sb[mt][:, it * P : (it + 1) * P],
                            woc_sb[:, mt, nh * SC : (nh + 1) * SC],
                            start=(mt == 0),
                            stop=(mt == 1),
                        )
                    dst = ys[:, nh * SC : (nh + 1) * SC]
                    if fast:
                        force = None if nh == 0 else (
                            "D" if engines[0] == "A" else "A")
                        e = pick(c_act(SC), c_dve(SC), force=force)
                    else:
                        e = pick(c_act(SC), c_dve(SC))
                    engines.append(e)
                    if e == "A":
                        nc.scalar.activation(dst, yp, COPY)
                    else:
                        nc.vector.tensor_copy(dst, yp)
                    nc.sync.dma_start(
                        out=y.ap()[it * P : (it + 1) * P, nh * SC : (nh + 1) * SC],
                        in_=dst,
                    )

            work = deque()
            light = deque()
            slow = deque()  # deferred PE work, drained 1 per 2 ticks to fill
            # the attention-only phase where scores+PV alone under-fill the PE

            def drain(tick):
                if light:
                    light.popleft()()
                if work:
                    work.popleft()()
                elif slow and tick % 2 == 0:
                    slow.popleft()()

            def drain_until(key):
                while key not in emitted:
                    assert work or light or slow, f"work exhausted before {key}"
                    if light:
                        light.popleft()()
                    elif work:
                        work.popleft()()
                    else:
                        slow.popleft()()

            # ---- pipeline fill: K/Q/V chunk 0 ----
            # DMA order = need order: xk-h0, wq-mt0, xq-h0/h1 ahead of
            # everything else so score tick 0 fires as early as possible.
            dma_half("k", xk_t, 0, 0)
            proj_qk("k", 0, 0, 0, wk_sb, bk_sb, kt_sb, 1.0)
            nc.sync.dma_start(out=wq_sb[:, :, 0:P], in_=wq_ap[:, :, 0:P])
            dma_half("q", xq_t, 0, 0)
            dma_half("q", xq_t, 0, 1)
            emitted.add(("dma", "q", 0))
            proj_qk("q", 0, 0, 0, wq_sb, bq_sb, qt_sb, KPRE)
            proj_qk("q", 0, 0, 1, wq_sb, bq_sb, qt_sb, KPRE)
            dma_half("k", xk_t, 0, 1)
            emitted.add(("dma", "k", 0))
            nc.sync.dma_start(out=wk_sb[:, :, P:DC], in_=wk_ap[:, :, P:DC])
            nc.sync.dma_start(out=wq_sb[:, :, P:DC], in_=wq_ap[:, :, P:DC])
            nc.sync.dma_start(out=wv_sb, in_=wv_t.ap().rearrange("(c p) m -> p c m", p=P))
            dma_slab("v", xv_t, 0)

            def load_woc():
                nc.sync.dma_start(
                    out=woc_sb, in_=wo_t.ap().rearrange("(t p) n -> p t n", p=P)
                )

            work.append(lambda: proj_qk("k", 0, 0, 1, wk_sb, bk_sb, kt_sb, 1.0))
            work.append(lambda: proj_qk("k", 0, 1, 0, wk_sb, bk_sb, kt_sb, 1.0))
            work.append(lambda: proj_qk("k", 0, 1, 1, wk_sb, bk_sb, kt_sb, 1.0))
            work.append(lambda: proj_qk("q", 0, 1, 0, wq_sb, bq_sb, qt_sb, KPRE))
            work.append(lambda: proj_qk("q", 0, 1, 1, wq_sb, bq_sb, qt_sb, KPRE))
            for jc in range(4):
                work.append(lambda jc=jc: proj_v(jc))
            # K slabs lead V slabs in the DMA queue: ib0's score stream
            # consumes K chunks at nearly full DMA bandwidth
            light.append(lambda: dma_slab("k", xk_t, 1))
            light.append(lambda: dma_slab("v", xv_t, 1))
            light.append(lambda: dma_slab("k", xk_t, 2))
            light.append(lambda: dma_slab("k", xk_t, 3))
            light.append(lambda: dma_slab("v", xv_t, 2))
            light.append(lambda: dma_slab("v", xv_t, 3))
            for cb in range(1, NSC):
                for mt in range(2):
                    for half in range(2):
                        work.append(
                            lambda cb=cb, mt=mt, half=half: proj_qk(
                                "k", cb, mt, half, wk_sb, bk_sb, kt_sb, 1.0)
                        )
                for jc in range(cb * 4, cb * 4 + 4):
                    work.append(lambda jc=jc: proj_v(jc))
            light.append(load_woc)
            for cb in range(1, NSC):
                light.append(lambda cb=cb: dma_slab("q", xq_t, cb))
                for half in range(2):
                    slow.append(
                        lambda cb=cb, half=half: proj_qk(
                            "q", cb, 0, half, wq_sb, bq_sb, qt_sb, KPRE)
                    )
                    slow.append(
                        lambda cb=cb, half=half: proj_qk(
                            "q", cb, 1, half, wq_sb, bq_sb, qt_sb, KPRE)
                    )

            # ---- attention ticks: (ib, hp, J); head pair hp, 128-row j chunk J
            seq = [(ib, hp, J) for ib in range(NSC) for hp in range(2) for J in range(NJ)]
            u_tiles = {}
            et_tiles = {}

            def emit_st_exp(idx):
                ib, hp, J = seq[idx]
                if J == 0:
                    drain_until(("q", ib, hp, 0))
                    drain_until(("q", ib, hp, 1))
                if ib == 0:
                    drain_until(("k", J // 4, hp, (J % 4) // 2))
                st = stp.tile([P, 2 * SC], F32, tag="st", name="st")
                for hx in range(2):
                    nc.tensor.matmul(
                        st[:, hx * SC : (hx + 1) * SC],
                        kt_sb[hp][hx * DK : (hx + 1) * DK, J * P : (J + 1) * P],
                        qt_sb[hp][hx * DK : (hx + 1) * DK, ib * SC : (ib + 1) * SC],
                        start=True,
                        stop=True,
                        tile_position=(hx * DK, 0),
                    )
                et = etp.tile([P, 2, SC], I16, tag="et", name="et")
                # hx-split exp: ACT takes head hx0 (exact fp16 exp), DVE takes
                # hx1 (fused exp2 approx) CONCURRENTLY -- halves the
                # scores->exp->st-slot-free serial chain vs alternating whole
                # ticks between the engines.
                pick(c_act(SC), 0.0, force="A")
                pick(0.0, c_dve(SC), force="D")
                nc.scalar.activation(
                    et[:, 0, :].bitcast(FP16), st[:, 0:SC], EXP, scale=LN2_1024
                )
                nc.vector._custom_dve(
                    exp2_op,
                    out=et[:, 1, :],
                    in0=st[:, SC : 2 * SC],
                    in1=expc_sb,
                    s0=float(EXP_BIAS),
                    s1=EXP_MAGIC,
                    imm2=float(EXPC),
                )
                et_tiles[idx] = et

            def emit_pv(idx):
                ib, hp, J = seq[idx]
                if J == 0:
                    for hx in range(2):
                        u_tiles[(hp, hx)] = ub.tile([P, 4, 65], F32, tag="u", name="u")
                if ib == 0 and hp == 0:
                    drain_until(("v", J))
                et = et_tiles.pop(idx).bitcast(FP16)
                for hx in range(2):
                    u = u_tiles[(hp, hx)]
                    for it in range(4):
                        nc.tensor.matmul(
                            u[:, it, :],
                            et[:, hx, it * P : (it + 1) * P],
                            vaug[:, J, 2 * hp + hx, :],
                            start=(J == 0 and it == 0),
                            stop=(J == NJ - 1 and it == 3),
                            skip_group_check=True,
                            tile_position=(0, 0),
                        )
                if J == NJ - 1:
                    finish_pair(ib, hp)

            def finish_pair(ib, hp):
                last = ib == NSC - 1
                for hx in range(2):
                    u = u_tiles.pop((hp, hx))
                    h = 2 * hp + hx
                    rz = rzp.tile([P, 4, 1], F32, tag="rz", name="rz")
                    nc.vector.reciprocal(rz, u[:, :, 64:65])
                    ew["D"] += 4 * 1.042 + 170
                    for it in range(4):
                        dst = o_sb[:, ib * 4 + it, h * DK : (h + 1) * DK]
                        if last:
                            e = pick(c_act(DK), c_dve(DK),
                                     force="A" if (it + hx) % 2 == 0 else "D")
                        else:
                            e = pick(c_act(DK), c_dve(DK))
                        if e == "A":
                            nc.scalar.activation(dst, u[:, it, 0:DK], COPY, scale=rz[:, it])
                        else:
                            nc.vector.tensor_scalar(dst, u[:, it, 0:DK], rz[:, it], None, op0=MULT)
                # o -> ot transposes are XBAR DMAs: issue as soon as this
                # head-pair's o columns are written (mt == hp)
                if not (last and hp == 1):
                    light.extend(
                        [lambda pr=pr: transp2(ib, hp, pr) for pr in range(2)]
                    )
                if hp == 1:
                    if ib < NSC - 1:
                        slow.extend(
                            [lambda it=it: oproj(it) for it in range(ib * 4, ib * 4 + 4)]
                        )
                    else:
                        # tail: PE transposes + oproj interleaved per pr pair
                        for pr in range(2):
                            work.append(lambda pr=pr: transp2(ib, 1, pr, fast=True))
                            for it in range(ib * 4 + 2 * pr, ib * 4 + 2 * pr + 2):
                                work.append(lambda it=it: oproj(it, fast=True))

            SKEW = 4  # PV lags scores/exp so its matmuls never park in the
            # PE wait queue (depth 4) blocking later scores
            for idx in range(len(seq) + SKEW):
                if idx < len(seq):
                    emit_st_exp(idx)
                if idx >= SKEW:
                    emit_pv(idx - SKEW)
                drain(idx)

            while work or light or slow:
                (light or work or slow).popleft()()

    nc.compile()
    return nc


def _get_nc():
    global _NC_CACHE
    if _NC_CACHE is None:
        _NC_CACHE = _build()
    return _NC_CACHE


def _in_maps(query, key, value, wq, wk, wv, wo, bq, bk):
    bf = ml_dtypes.bfloat16
    maps = []
    for c in range(8):
        b, g = divmod(c, 4)
        sl = slice(g * DC, (g + 1) * DC)
        maps.append(
            {
                "xq_t": np.ascontiguousarray(query[:, b, :].T).astype(bf),
                "xk_t": np.ascontiguousarray(key[:, b, :].T).astype(bf),
                "xv_t": np.ascontiguousarray(value[:, b, :].T).astype(bf),
                "wq_t": np.ascontiguousarray(wq[sl, :].T).astype(bf),
                "wk_t": np.ascontiguousarray(wk[sl, :].T).astype(bf),
                "wv_t": np.ascontiguousarray(wv[sl, :].T).astype(bf),
                "wo_t": np.ascontiguousarray(wo[:, sl].T).astype(bf),
                "bq_s": np.ascontiguousarray(
                    (bq[sl] * KPRE).astype(np.float32).reshape(2, P).T
                ),
                "bk_s": np.ascontiguousarray(bk[sl].reshape(2, P).T),
            }
        )
    return maps


def kernel(
    query, key, value, wq, bq, wk, bk, wv, bv, wo, bo, **_kw
) -> np.ndarray:
    query = np.asarray(query, np.float32)
    key = np.asarray(key, np.float32)
    value = np.asarray(value, np.float32)
    wq = np.asarray(wq, np.float32)
    wk = np.asarray(wk, np.float32)
    wv = np.asarray(wv, np.float32)
    wo = np.asarray(wo, np.float32)
    bq = np.asarray(bq, np.float32)
    bk = np.asarray(bk, np.float32)
    bv = np.asarray(bv, np.float32)
    bo = np.asarray(bo, np.float32)

    nc = _get_nc()
    res = run_bass_kernel_spmd(
        nc, _in_maps(query, key, value, wq, wk, wv, wo, bq, bk),
        core_ids=list(range(8)),
    )

    out = np.zeros((S, B, D), np.float32)
    for c in range(8):
        out[:, c // 4, :] += res.results[c]["y"].astype(np.float32)
    out += bo + wo @ bv
    return out


# revision 26
# speedup vs baseline: 1.0703x; 1.0703x over previous
"""Multi-head attention (S=2048, B=2, D=1024, H=16) on 8 Trainium2 NeuronCores.

Sharding: batch x head-group. Core c handles batch c//4 and heads
[4*(c%4), 4*(c%4)+4). Each core computes its 4 heads' Q/K/V projections,
attention, and a partial output projection (row-parallel Wo); the host sums
the 4 partials per batch and adds the bias terms (bo and the exact wo@bv
correction; softmax rows sum to 1 so bv folds out of the attention).

v4 device-side structure (per core):
  - x inputs and all weights arrive as bf16 (host-converted), halving HBM
    traffic; projections run bf16 through the PE at 1 cycle/row.
  - Q is projected PRE-SCALED by K' = SCALE*1024/ln2, so score PSUM tiles
    hold T = s*SCALE*1024/ln2 directly (the fp16 exponent-unit domain).
  - exp splits across ACT and DVE per tick parity:
      ACT: activation(Exp, scale=ln2/1024) -> exact fp16.
      DVE: one fused custom op (EXP2_FUSED_ANT, 8 ALU stages): magic-number
      floor-to-1024 grid, quadratic mantissa correction
      mant = F + c*F*(F-1024), c = 0.34/1024 -> int16 bit pattern that IS
      fp16(exp(s*SCALE)) with ~0.3% max error (vs ~2% Schraudolph).
  - All other elementwise work (bias adds, conversions, 1/Z normalize,
    stage copies) is greedily load-balanced across ACT, DVE, and Pool
    (GPSIMD), keeping ACT/DVE exp throughput ahead of the PE.
  - P@V runs fp16 with a ones-column appended to V so the PE accumulates
    the softmax denominator Z next to U; normalization is deferred past
    P@V (one reciprocal + per-partition-scalar multiplies).
  - PSUM is partitioned to decouple the pipeline: score tiles get a
    dedicated 2-deep rotation (4 banks), the two live U accumulators get
    their own banks, and projection/transpose/output-projection scratch
    rotates in a third pool, so scores(t+1) never serialize behind exp(t).
  - P@V is emitted four ticks behind scores/exp so its matmuls never park
    in the PE wait queue (depth 4) blocking later scores; input slabs land
    as two half-DMAs so projections start after half the transfer.
  - O = U * (1/Z) -> bf16, PE-transposed, output projection in bf16;
    y partials fly to HBM as bf16 and the host sums them in f32.
"""

import sys

sys.path.insert(0, "/opt/trn_rl_repo")

from collections import deque

import numpy as np
import ml_dtypes

import concourse.bass as bass
import concourse.tile as tile
from concourse import bacc, mybir
from concourse.bass_utils import run_bass_kernel_spmd
from concourse.masks import make_identity


S = 2048
B = 2
D = 1024
H = 16
DK = 64
G = 4            # heads per core
DC = G * DK      # 256 per-core head dims
SCALE = 1.0 / np.sqrt(DK)  # 0.125
P = 128
NSC = 4          # 512-col s-chunks (query i-blocks and proj chunks)
SC = S // NSC    # 512
NJ = 16          # 128-row j chunks
ND = D // P      # 8 contraction chunks for projections

F32 = mybir.dt.float32
F32R = mybir.dt.float32r
BF16 = mybir.dt.bfloat16
I16 = mybir.dt.int16
FP16 = mybir.dt.float16
EXP = mybir.ActivationFunctionType.Exp
IDENT = mybir.ActivationFunctionType.Identity
COPY = mybir.ActivationFunctionType.Copy
ADD = mybir.AluOpType.add
MULT = mybir.AluOpType.mult

# Q pre-scale: score PSUM holds T = s * SCALE * 1024 / ln2
KPRE = float(SCALE * 1024.0 / np.log(2.0))
LN2_1024 = float(np.log(2.0) / 1024.0)
EXPC = 0.34 / 1024.0         # quadratic mantissa-correction constant
EXP_BIAS = 15360.0           # fp16 exponent bias in mantissa counts (15<<10)
EXP_MAGIC = float(2.0**33 - 512.0)  # f32 round-to-1024-grid magic (exact)
EXPC_SHIFT = float(-EXPC * 262144.0)  # -c*512^2, via the C3/Src1 latch

_NC_CACHE = None
_EXP2_OP = None


def _get_exp2_op():
    """Fused DVE op: in0 = T (f32); out int16 = fp16 bit pattern of
    2^(T/1024) with quadratic mantissa correction.

      T3 = T + B;  r = (T3+C1)-C1, C1 = M-512   [floor-to-1024 grid]
      Ft = T3 - r = F - 512  in [-512, 512)
      y  = (T3 - c*512^2) + c*Ft^2   [= T + B + c*F*(F-1024)] -> round -> u16

    Scalars: s0 = B (=15360), s1 = M-512, imm2 = c; in1 = [P,1] tile
    holding -c*512^2 (delivered via the C3/Src1 latch).
    """
    global _EXP2_OP
    if _EXP2_OP is not None:
        return _EXP2_OP
    from concourse import dve_ops
    from concourse.dve_spec import Spec, Src0, C0, C1, C2, C3, lower
    from concourse.dve_ops import DveOp, _spill_c3_to_src1, has_src1
    from concourse.dve_uop import DveOpSpec

    name = "EXP2_FUSED_ANT"
    T3 = Src0 + C0
    u = T3 + C1
    r = u - C1
    Ft = T3 - r
    Q2 = Ft * Ft
    Pn = Q2 * C2
    T3b = T3 + C3
    body = _spill_c3_to_src1(T3b + Pn)

    def _ref(in0, in1, s0, s1, imm2):
        f = np.float32
        cq = np.asarray(in1, f).reshape(in1.shape[0], -1)[:, :1]
        T3 = (np.asarray(in0, f) + f(s0)).astype(f)
        u = (T3 + f(s1)).astype(f)
        r = (u - f(s1)).astype(f)
        Ft = (T3 - r).astype(f)
        Q2 = (Ft * Ft).astype(f)
        Pn = (Q2 * f(imm2)).astype(f)
        T3b = (T3 + cq).astype(f)
        y = (T3b + Pn).astype(f)
        return np.rint(y).astype(f)

    spec = Spec(body=body, reference=_ref)
    row = 17
    dve_ops._SUB_OPCODE_FOR_NAME[name] = row
    shas = {}
    for ver in ("v3", "v4"):
        try:
            s = DveOpSpec(
                name=name, opcode=row, uops=lower(spec, ver=ver),
                rd1_en=has_src1(spec),
            )
            shas[ver] = s.sha(ver)
        except Exception:
            pass
    op = DveOp(name, spec, subdim=False, uops_sha=shas)
    dve_ops.OPS.append(op)
    dve_ops.CUSTOM_DVE_SPECS[name] = spec
    _EXP2_OP = op
    return op


def _build():
    nc = bacc.Bacc("TRN2", target_bir_lowering=False, debug=False)
    exp2_op = _get_exp2_op()

    xq_t = nc.dram_tensor("xq_t", [D, S], BF16, kind="ExternalInput")
    xk_t = nc.dram_tensor("xk_t", [D, S], BF16, kind="ExternalInput")
    xv_t = nc.dram_tensor("xv_t", [D, S], BF16, kind="ExternalInput")
    wq_t = nc.dram_tensor("wq_t", [D, DC], BF16, kind="ExternalInput")
    wk_t = nc.dram_tensor("wk_t", [D, DC], BF16, kind="ExternalInput")
    wv_t = nc.dram_tensor("wv_t", [D, DC], BF16, kind="ExternalInput")
    wo_t = nc.dram_tensor("wo_t", [DC, D], BF16, kind="ExternalInput")
    bq_s = nc.dram_tensor("bq_s", [P, 2], F32, kind="ExternalInput")  # pre-scaled by KPRE
    bk_s = nc.dram_tensor("bk_s", [P, 2], F32, kind="ExternalInput")
    y = nc.dram_tensor("y", [S, D], BF16, kind="ExternalOutput")

    # greedy two-engine load balancer for elementwise work (Pool/GPSIMD
    # cannot access PSUM, and every elementwise op here reads PSUM).
    ew = {"A": 0.0, "D": 0.0}

    def pick(cost_a, cost_d, force=None):
        if force is None:
            e = "A" if ew["A"] + cost_a <= ew["D"] + cost_d else "D"
        else:
            e = force
        ew[e] += cost_a if e == "A" else cost_d
        return e

    def c_act(n):  # ACT engine busy ns for n-col op
        return n * 0.833 + 185
    def c_dve(n, half=False):
        return n * (0.521 if half else 1.042) + 125

    with tile.TileContext(nc) as tc:
        with (
            tc.tile_pool(name="persist", bufs=1) as persist,
            tc.tile_pool(name="xs", bufs=8) as xs,
            tc.tile_pool(name="stp", bufs=2, space="PSUM") as stp,
            tc.tile_pool(name="ub", bufs=2, space="PSUM") as ub,
            tc.tile_pool(name="scr", bufs=2, space="PSUM") as scr,
            tc.tile_pool(name="et", bufs=12) as etp,
            tc.tile_pool(name="rz", bufs=4) as rzp,
            tc.tile_pool(name="ysb", bufs=6) as ysb,
        ):
            # ---- persistent SBUF ----
            wq_sb = persist.tile([P, ND, DC], BF16)
            wk_sb = persist.tile([P, ND, DC], BF16)
            wv_sb = persist.tile([P, ND, DC], BF16)
            bq_sb = persist.tile([P, 2], F32)
            bk_sb = persist.tile([P, 2], F32)
            expc_sb = persist.tile([P, 1], F32)
            wk_ap = wk_t.ap().rearrange("(c p) m -> p c m", p=P)
            wq_ap = wq_t.ap().rearrange("(c p) m -> p c m", p=P)
            # mt0 column-halves of Wk/Wq land first so the first projection
            # chain (k-mt0-h0 -> q-mt0-h0/h1 -> score tick 0) starts ~5us
            # earlier than with full-weight transfers
            nc.sync.dma_start(out=wk_sb[:, :, 0:P], in_=wk_ap[:, :, 0:P])
            nc.sync.dma_start(out=bk_sb, in_=bk_s.ap())
            nc.sync.dma_start(out=bq_sb, in_=bq_s.ap())
            nc.gpsimd.memset(expc_sb, float(EXPC_SHIFT))
            woc_sb = persist.tile([P, 2, D], BF16)

            qt_sb = [persist.tile([P, S], F32R, name=f"qt{t}") for t in range(2)]
            kt_sb = [persist.tile([P, S], F32R, name=f"kt{t}") for t in range(2)]
            # vaug[p, jc, h, d]; d==64 is the ones column for Z
            vaug = persist.tile([P, NJ, G, 65], FP16)
            nc.vector.memset(vaug[:, :, :, 64:65], 1.0)
            o_sb = persist.tile([P, NSC * 4, DC], BF16)
            ot_sb = [persist.tile([P, S], BF16, name=f"ot{t}") for t in range(2)]
            ident_f = persist.tile([P, P], F32)
            make_identity(nc, ident_f)
            ident = persist.tile([P, P], BF16)
            nc.vector.tensor_copy(ident, ident_f)

            # ---- work items ----
            slabs = {}
            emitted = set()

            def dma_half(key, xt, cb, hf):
                # one 256-col s-half of an input slab: consumers of the first
                # half start after half the transfer latency (subtile deps)
                if (key, cb) not in slabs:
                    slabs[(key, cb)] = xs.tile([P, ND, SC], BF16, tag="x", name="x")
                t = slabs[(key, cb)]
                src_ap = xt.ap().rearrange("(c p) s -> p c s", p=P)
                nc.sync.dma_start(
                    out=t[:, :, hf * HW_ : (hf + 1) * HW_],
                    in_=src_ap[
                        :, :, cb * SC + hf * HW_ : cb * SC + (hf + 1) * HW_
                    ],
                )

            def dma_slab(key, xt, cb):
                for hf in range(2):
                    dma_half(key, xt, cb, hf)
                emitted.add(("dma", key, cb))

            HW_ = SC // 2  # 256-col half for short PSUM holds

            def proj_qk(key, cb, mt, half, w_sb, b_sb, out_tiles, scale):
                ps = scr.tile([P, HW_], F32, tag="scr", name="ps")
                slab = slabs[(key, cb)]
                for dc in range(ND):
                    nc.tensor.matmul(
                        ps,
                        w_sb[:, dc, mt * P : (mt + 1) * P],
                        slab[:, dc, half * HW_ : (half + 1) * HW_],
                        start=(dc == 0),
                        stop=(dc == ND - 1),
                    )
                dst = out_tiles[mt][:, cb * SC + half * HW_ : cb * SC + (half + 1) * HW_]
                e = pick(c_act(HW_), c_dve(HW_))
                if e == "A":
                    nc.scalar.activation(
                        dst, ps, IDENT, bias=b_sb[:, mt : mt + 1], scale=scale
                    )
                else:
                    nc.vector.tensor_scalar(
                        dst, ps, scale, b_sb[:, mt : mt + 1], op0=MULT, op1=ADD
                    )
                emitted.add((key, cb, mt, half))

            def proj_v(jc):
                # j chunk jc (128 rows) -> vaug[:, jc, :, 0:64]
                cb, jq = divmod(jc, 4)
                ps = scr.tile([P, DC], F32, tag="scr", name="ps")
                slab = slabs[("v", cb)]
                for dc in range(ND):
                    nc.tensor.matmul(
                        ps,
                        slab[:, dc, jq * P : (jq + 1) * P],
                        wv_sb[:, dc, :],
                        start=(dc == 0),
                        stop=(dc == ND - 1),
                    )
                dst = vaug[:, jc, :, 0:64]
                src = ps.rearrange("p (h c) -> p h c", h=G)
                e = pick(c_act(DC), c_dve(DC))
                if e == "A":
                    nc.scalar.activation(dst, src, COPY)
                else:
                    nc.vector.tensor_copy(dst, src)
                emitted.add(("v", jc))

            def transp2(ib, mt, pr, fast=False):
                # XBAR DMA transpose (2-byte, SBUF->SBUF): no PE or ACT/DVE
                # work at all; o_sb [i, dc] bf16 -> ot_sb [dc, i].
                # fast=True (kernel tail): PE transpose + A/D copy instead --
                # the PE is idle there and XBAR DMA latency (~1.5us) would
                # sit on the critical path.
                for it in range(ib * 4 + 2 * pr, ib * 4 + 2 * pr + 2):
                    if fast:
                        tp = scr.tile([P, P], BF16, tag="scr", name="tp")
                        nc.tensor.transpose(tp, o_sb[:, it, mt * P : (mt + 1) * P], ident)
                        dst = ot_sb[mt][:, it * P : (it + 1) * P]
                        e = pick(c_act(P), c_dve(P, half=True))
                        if e == "A":
                            nc.scalar.activation(dst, tp, COPY)
                        else:
                            nc.vector.tensor_copy(dst, tp)
                    else:
                        nc.sync.dma_start(
                            out=ot_sb[mt][:, it * P : (it + 1) * P],
                            in_=o_sb[:, it, mt * P : (mt + 1) * P],
                            transpose=True,
                        )

            def oproj(it, fast=False):
                # each half flies to HBM right after its stage copy, and the
                # two copies go to different engines: halves the tail chain
                ys = ysb.tile([P, D], BF16, tag="ysb", name="ysb")
                engines = []
                for nh in range(2):
                    yp = scr.tile([P, SC], F32, tag="scr", name="yp")
                    for mt in range(2):
                        nc.tensor.matmul(
                            yp,
                            ot_sb[mt][:, it * P : (it + 1) * P],
                            woc_sb[:, mt, nh * SC : (nh + 1) * SC],
                            start=(mt == 0),
                            stop=(mt == 1),
                        )
                    dst = ys[:, nh * SC : (nh + 1) * SC]
                    if fast:
                        force = None if nh == 0 else (
                            "D" if engines[0] == "A" else "A")
                        e = pick(c_act(SC), c_dve(SC), force=force)
                    else:
                        e = pick(c_act(SC), c_dve(SC))
                    engines.append(e)
                    if e == "A":
                        nc.scalar.activation(dst, yp, COPY)
                    else:
                        nc.vector.tensor_copy(dst, yp)
                    nc.sync.dma_start(
                        out=y.ap()[it * P : (it + 1) * P, nh * SC : (nh + 1) * SC],
                        in_=dst,
                    )

            work = deque()
            light = deque()
            slow = deque()  # deferred PE work, drained 1 per 2 ticks to fill
            # the attention-only phase where scores+PV alone under-fill the PE

            def drain(tick):
                if light:
                    light.popleft()()
                if work:
                    work.popleft()()
                elif slow and tick % 2 == 0:
                    slow.popleft()()

            def drain_until(key):
                while key not in emitted:
                    assert work or light or slow, f"work exhausted before {key}"
                    if light:
                        light.popleft()()
                    elif work:
                        work.popleft()()
                    else:
                        slow.popleft()()

            # ---- pipeline fill: K/Q/V chunk 0 ----
            # DMA order = need order: xk-h0, wq-mt0, xq-h0/h1 ahead of
            # everything else so score tick 0 fires as early as possible.
            dma_half("k", xk_t, 0, 0)
            proj_qk("k", 0, 0, 0, wk_sb, bk_sb, kt_sb, 1.0)
            nc.sync.dma_start(out=wq_sb[:, :, 0:P], in_=wq_ap[:, :, 0:P])
            dma_half("q", xq_t, 0, 0)
            dma_half("q", xq_t, 0, 1)
            emitted.add(("dma", "q", 0))
            proj_qk("q", 0, 0, 0, wq_sb, bq_sb, qt_sb, KPRE)
            proj_qk("q", 0, 0, 1, wq_sb, bq_sb, qt_sb, KPRE)
            dma_half("k", xk_t, 0, 1)
            emitted.add(("dma", "k", 0))
            nc.sync.dma_start(out=wk_sb[:, :, P:DC], in_=wk_ap[:, :, P:DC])
            nc.sync.dma_start(out=wq_sb[:, :, P:DC], in_=wq_ap[:, :, P:DC])
            nc.sync.dma_start(out=wv_sb, in_=wv_t.ap().rearrange("(c p) m -> p c m", p=P))
            dma_slab("v", xv_t, 0)

            def load_woc():
                nc.sync.dma_start(
                    out=woc_sb, in_=wo_t.ap().rearrange("(t p) n -> p t n", p=P)
                )

            work.append(lambda: proj_qk("k", 0, 0, 1, wk_sb, bk_sb, kt_sb, 1.0))
            work.append(lambda: proj_qk("k", 0, 1, 0, wk_sb, bk_sb, kt_sb, 1.0))
            work.append(lambda: proj_qk("k", 0, 1, 1, wk_sb, bk_sb, kt_sb, 1.0))
            work.append(lambda: proj_qk("q", 0, 1, 0, wq_sb, bq_sb, qt_sb, KPRE))
            work.append(lambda: proj_qk("q", 0, 1, 1, wq_sb, bq_sb, qt_sb, KPRE))
            for jc in range(4):
                work.append(lambda jc=jc: proj_v(jc))
            # K slabs lead V slabs in the DMA queue: ib0's score stream
            # consumes K chunks at nearly full DMA bandwidth
            light.append(lambda: dma_slab("k", xk_t, 1))
            light.append(lambda: dma_slab("v", xv_t, 1))
            light.append(lambda: dma_slab("k", xk_t, 2))
            light.append(lambda: dma_slab("k", xk_t, 3))
            light.append(lambda: dma_slab("v", xv_t, 2))
            light.append(lambda: dma_slab("v", xv_t, 3))
            for cb in range(1, NSC):
                for mt in range(2):
                    for half in range(2):
                        work.append(
                            lambda cb=cb, mt=mt, half=half: proj_qk(
                                "k", cb, mt, half, wk_sb, bk_sb, kt_sb, 1.0)
                        )
                for jc in range(cb * 4, cb * 4 + 4):
                    work.append(lambda jc=jc: proj_v(jc))
            light.append(load_woc)
            for cb in range(1, NSC):
                light.append(lambda cb=cb: dma_slab("q", xq_t, cb))
                for half in range(2):
                    slow.append(
                        lambda cb=cb, half=half: proj_qk(
                            "q", cb, 0, half, wq_sb, bq_sb, qt_sb, KPRE)
                    )
                    slow.append(
                        lambda cb=cb, half=half: proj_qk(
                            "q", cb, 1, half, wq_sb, bq_sb, qt_sb, KPRE)
                    )

            # ---- attention ticks: (ib, hp, J); head pair hp, 128-row j chunk J
            seq = [(ib, hp, J) for ib in range(NSC) for hp in range(2) for J in range(NJ)]
            u_tiles = {}
            et_tiles = {}

            def emit_st_exp(idx):
                ib, hp, J = seq[idx]
                if J == 0:
                    drain_until(("q", ib, hp, 0))
                    drain_until(("q", ib, hp, 1))
                if ib == 0:
                    drain_until(("k", J // 4, hp, (J % 4) // 2))
                st = stp.tile([P, 2 * SC], F32, tag="st", name="st")
                for hx in range(2):
                    nc.tensor.matmul(
                        st[:, hx * SC : (hx + 1) * SC],
                        kt_sb[hp][hx * DK : (hx + 1) * DK, J * P : (J + 1) * P],
                        qt_sb[hp][hx * DK : (hx + 1) * DK, ib * SC : (ib + 1) * SC],
                        start=True,
                        stop=True,
                        tile_position=(hx * DK, 0),
                    )
                et = etp.tile([P, 2, SC], I16, tag="et", name="et")
                et_flat = et.rearrange("p a b -> p (a b)")
                # strict A/D alternation keeps the two st pipelines decoupled
                e = pick(c_act(2 * SC), c_dve(2 * SC),
                         force="A" if idx % 2 == 0 else "D")
                if e == "A":
                    nc.scalar.activation(
                        et_flat.bitcast(FP16), st, EXP, scale=LN2_1024
                    )
                else:
                    nc.vector._custom_dve(
                        exp2_op,
                        out=et_flat,
                        in0=st,
                        in1=expc_sb,
                        s0=float(EXP_BIAS),
                        s1=EXP_MAGIC,
                        imm2=float(EXPC),
                    )
                et_tiles[idx] = et

            def emit_pv(idx):
                ib, hp, J = seq[idx]
                if J == 0:
                    for hx in range(2):
                        u_tiles[(hp, hx)] = ub.tile([P, 4, 65], F32, tag="u", name="u")
                if ib == 0 and hp == 0:
                    drain_until(("v", J))
                et = et_tiles.pop(idx).bitcast(FP16)
                for hx in range(2):
                    u = u_tiles[(hp, hx)]
                    for it in range(4):
                        nc.tensor.matmul(
                            u[:, it, :],
                            et[:, hx, it * P : (it + 1) * P],
                            vaug[:, J, 2 * hp + hx, :],
                            start=(J == 0 and it == 0),
                            stop=(J == NJ - 1 and it == 3),
                            skip_group_check=True,
                            tile_position=(0, 0),
                        )
                if J == NJ - 1:
                    finish_pair(ib, hp)

            def finish_pair(ib, hp):
                last = ib == NSC - 1
                for hx in range(2):
                    u = u_tiles.pop((hp, hx))
                    h = 2 * hp + hx
                    rz = rzp.tile([P, 4, 1], F32, tag="rz", name="rz")
                    nc.vector.reciprocal(rz, u[:, :, 64:65])
                    ew["D"] += 4 * 1.042 + 170
                    for it in range(4):
                        dst = o_sb[:, ib * 4 + it, h * DK : (h + 1) * DK]
                        if last:
                            e = pick(c_act(DK), c_dve(DK),
                                     force="A" if (it + hx) % 2 == 0 else "D")
                        else:
                            e = pick(c_act(DK), c_dve(DK))
                        if e == "A":
                            nc.scalar.activation(dst, u[:, it, 0:DK], COPY, scale=rz[:, it])
                        else:
                            nc.vector.tensor_scalar(dst, u[:, it, 0:DK], rz[:, it], None, op0=MULT)
                # o -> ot transposes are XBAR DMAs: issue as soon as this
                # head-pair's o columns are written (mt == hp)
                if not (last and hp == 1):
                    light.extend(
                        [lambda pr=pr: transp2(ib, hp, pr) for pr in range(2)]
                    )
                if hp == 1:
                    if ib < NSC - 1:
                        slow.extend(
                            [lambda it=it: oproj(it) for it in range(ib * 4, ib * 4 + 4)]
                        )
                    else:
                        # tail: PE transposes + oproj interleaved per pr pair
                        for pr in range(2):
                            work.append(lambda pr=pr: transp2(ib, 1, pr, fast=True))
                            for it in range(ib * 4 + 2 * pr, ib * 4 + 2 * pr + 2):
                                work.append(lambda it=it: oproj(it, fast=True))

            SKEW = 4  # PV lags scores/exp so its matmuls never park in the
            # PE wait queue (depth 4) blocking later scores
            for idx in range(len(seq) + SKEW):
                if idx < len(seq):
                    emit_st_exp(idx)
                if idx >= SKEW:
                    emit_pv(idx - SKEW)
                drain(idx)

            while work or light or slow:
                (light or work or slow).popleft()()

    nc.compile()
    return nc


def _get_nc():
    global _NC_CACHE
    if _NC_CACHE is None:
        _NC_CACHE = _build()
    return _NC_CACHE


def _in_maps(query, key, value, wq, wk, wv, wo, bq, bk):
    bf = ml_dtypes.bfloat16
    maps = []
    for c in range(8):
        b, g = divmod(c, 4)
        sl = slice(g * DC, (g + 1) * DC)
        maps.append(
            {
                "xq_t": np.ascontiguousarray(query[:, b, :].T).astype(bf),
                "xk_t": np.ascontiguousarray(key[:, b, :].T).astype(bf),
                "xv_t": np.ascontiguousarray(value[:, b, :].T).astype(bf),
                "wq_t": np.ascontiguousarray(wq[sl, :].T).astype(bf),
                "wk_t": np.ascontiguousarray(wk[sl, :].T).astype(bf),
                "wv_t": np.ascontiguousarray(wv[sl, :].T).astype(bf),
                "wo_t": np.ascontiguousarray(wo[:, sl].T).astype(bf),
                "bq_s": np.ascontiguousarray(
                    (bq[sl] * KPRE).astype(np.float32).reshape(2, P).T
                ),
                "bk_s": np.ascontiguousarray(bk[sl].reshape(2, P).T),
            }
        )
    return maps


def kernel(
    query, key, value, wq, bq, wk, bk, wv, bv, wo, bo, **_kw
) -> np.ndarray:
    query = np.asarray(query, np.float32)
    key = np.asarray(key, np.float32)
    value = np.asarray(value, np.float32)
    wq = np.asarray(wq, np.float32)
    wk = np.asarray(wk, np.float32)
    wv = np.asarray(wv, np.float32)
    wo = np.asarray(wo, np.float32)
    bq = np.asarray(bq, np.float32)
    bk = np.asarray(bk, np.float32)
    bv = np.asarray(bv, np.float32)
    bo = np.asarray(bo, np.float32)

    nc = _get_nc()
    res = run_bass_kernel_spmd(
        nc, _in_maps(query, key, value, wq, wk, wv, wo, bq, bk),
        core_ids=list(range(8)),
    )

    out = np.zeros((S, B, D), np.float32)
    for c in range(8):
        out[:, c // 4, :] += res.results[c]["y"].astype(np.float32)
    out += bo + wo @ bv
    return out


# revision 39
# speedup vs baseline: 1.0740x; 1.0035x over previous
"""Multi-head attention (S=2048, B=2, D=1024, H=16) on 8 Trainium2 NeuronCores.

Sharding: batch x head-group. Core c handles batch c//4 and heads
[4*(c%4), 4*(c%4)+4). Each core computes its 4 heads' Q/K/V projections,
attention, and a partial output projection (row-parallel Wo); the host sums
the 4 partials per batch and adds the bias terms (bo and the exact wo@bv
correction; softmax rows sum to 1 so bv folds out of the attention).

v4 device-side structure (per core):
  - x inputs and all weights arrive as bf16 (host-converted), halving HBM
    traffic; projections run bf16 through the PE at 1 cycle/row.
  - Q is projected PRE-SCALED by K' = SCALE*1024/ln2, so score PSUM tiles
    hold T = s*SCALE*1024/ln2 directly (the fp16 exponent-unit domain).
  - exp splits across ACT and DVE per tick parity:
      ACT: activation(Exp, scale=ln2/1024) -> exact fp16.
      DVE: one fused custom op (EXP2_FUSED_ANT, 8 ALU stages): magic-number
      floor-to-1024 grid, quadratic mantissa correction
      mant = F + c*F*(F-1024), c = 0.34/1024 -> int16 bit pattern that IS
      fp16(exp(s*SCALE)) with ~0.3% max error (vs ~2% Schraudolph).
  - All other elementwise work (bias adds, conversions, 1/Z normalize,
    stage copies) is greedily load-balanced across ACT, DVE, and Pool
    (GPSIMD), keeping ACT/DVE exp throughput ahead of the PE.
  - P@V runs fp16 with a ones-column appended to V so the PE accumulates
    the softmax denominator Z next to U; normalization is deferred past
    P@V (one reciprocal + per-partition-scalar multiplies).
  - PSUM is partitioned to decouple the pipeline: score tiles get a
    dedicated 2-deep rotation (4 banks), the two live U accumulators get
    their own banks, and projection/transpose/output-projection scratch
    rotates in a third pool, so scores(t+1) never serialize behind exp(t).
  - P@V is emitted four ticks behind scores/exp so its matmuls never park
    in the PE wait queue (depth 4) blocking later scores; input slabs land
    as two half-DMAs so projections start after half the transfer.
  - O = U * (1/Z) -> bf16, PE-transposed, output projection in bf16;
    y partials fly to HBM as bf16 and the host sums them in f32.
"""

import sys

sys.path.insert(0, "/opt/trn_rl_repo")

from collections import deque

import numpy as np
import ml_dtypes

import concourse.bass as bass
import concourse.tile as tile
from concourse import bacc, mybir
from concourse.bass_utils import run_bass_kernel_spmd
from concourse.masks import make_identity


S = 2048
B = 2
D = 1024
H = 16
DK = 64
G = 4            # heads per core
DC = G * DK      # 256 per-core head dims
SCALE = 1.0 / np.sqrt(DK)  # 0.125
P = 128
NSC = 4          # 512-col s-chunks (projection chunks)
SC = S // NSC    # 512
NIB = 8          # 256-col query i-blocks (attention ticks)
IBW = S // NIB   # 256
NJ = 16          # 128-row j chunks
ND = D // P      # 8 contraction chunks for projections

F32 = mybir.dt.float32
F32R = mybir.dt.float32r
BF16 = mybir.dt.bfloat16
I16 = mybir.dt.int16
FP16 = mybir.dt.float16
EXP = mybir.ActivationFunctionType.Exp
IDENT = mybir.ActivationFunctionType.Identity
COPY = mybir.ActivationFunctionType.Copy
ADD = mybir.AluOpType.add
MULT = mybir.AluOpType.mult

# Q pre-scale: score PSUM holds T = s * SCALE * 1024 / ln2
KPRE = float(SCALE * 1024.0 / np.log(2.0))
LN2_1024 = float(np.log(2.0) / 1024.0)
EXPC = 0.34 / 1024.0         # quadratic mantissa-correction constant
EXP_BIAS = 15360.0           # fp16 exponent bias in mantissa counts (15<<10)
EXP_MAGIC = float(2.0**33 - 512.0)  # f32 round-to-1024-grid magic (exact)
EXPC_SHIFT = float(-EXPC * 262144.0)  # -c*512^2, via the C3/Src1 latch

_NC_CACHE = None
_EXP2_OP = None


def _get_exp2_op():
    """Fused DVE op: in0 = T (f32); out int16 = fp16 bit pattern of
    2^(T/1024) with quadratic mantissa correction.

      T3 = T + B;  r = (T3+C1)-C1, C1 = M-512   [floor-to-1024 grid]
      Ft = T3 - r = F - 512  in [-512, 512)
      y  = (T3 - c*512^2) + c*Ft^2   [= T + B + c*F*(F-1024)] -> round -> u16

    Scalars: s0 = B (=15360), s1 = M-512, imm2 = c; in1 = [P,1] tile
    holding -c*512^2 (delivered via the C3/Src1 latch).
    """
    global _EXP2_OP
    if _EXP2_OP is not None:
        return _EXP2_OP
    from concourse import dve_ops
    from concourse.dve_spec import Spec, Src0, C0, C1, C2, C3, lower
    from concourse.dve_ops import DveOp, _spill_c3_to_src1, has_src1
    from concourse.dve_uop import DveOpSpec

    name = "EXP2_FUSED_ANT"
    T3 = Src0 + C0
    u = T3 + C1
    r = u - C1
    Ft = T3 - r
    Q2 = Ft * Ft
    Pn = Q2 * C2
    T3b = T3 + C3
    body = _spill_c3_to_src1(T3b + Pn)

    def _ref(in0, in1, s0, s1, imm2):
        f = np.float32
        cq = np.asarray(in1, f).reshape(in1.shape[0], -1)[:, :1]
        T3 = (np.asarray(in0, f) + f(s0)).astype(f)
        u = (T3 + f(s1)).astype(f)
        r = (u - f(s1)).astype(f)
        Ft = (T3 - r).astype(f)
        Q2 = (Ft * Ft).astype(f)
        Pn = (Q2 * f(imm2)).astype(f)
        T3b = (T3 + cq).astype(f)
        y = (T3b + Pn).astype(f)
        return np.rint(y).astype(f)

    spec = Spec(body=body, reference=_ref)
    row = 17
    dve_ops._SUB_OPCODE_FOR_NAME[name] = row
    shas = {}
    for ver in ("v3", "v4"):
        try:
            s = DveOpSpec(
                name=name, opcode=row, uops=lower(spec, ver=ver),
                rd1_en=has_src1(spec),
            )
            shas[ver] = s.sha(ver)
        except Exception:
            pass
    op = DveOp(name, spec, subdim=False, uops_sha=shas)
    dve_ops.OPS.append(op)
    dve_ops.CUSTOM_DVE_SPECS[name] = spec
    _EXP2_OP = op
    return op


def _build():
    nc = bacc.Bacc("TRN2", target_bir_lowering=False, debug=False)
    exp2_op = _get_exp2_op()

    xq_t = nc.dram_tensor("xq_t", [D, S], BF16, kind="ExternalInput")
    xk_t = nc.dram_tensor("xk_t", [D, S], BF16, kind="ExternalInput")
    xv_t = nc.dram_tensor("xv_t", [D, S], BF16, kind="ExternalInput")
    wq_t = nc.dram_tensor("wq_t", [D, DC], BF16, kind="ExternalInput")
    wk_t = nc.dram_tensor("wk_t", [D, DC], BF16, kind="ExternalInput")
    wv_t = nc.dram_tensor("wv_t", [D, DC], BF16, kind="ExternalInput")
    wo_t = nc.dram_tensor("wo_t", [DC, D], BF16, kind="ExternalInput")
    bq_s = nc.dram_tensor("bq_s", [P, 2], F32, kind="ExternalInput")  # pre-scaled by KPRE
    bk_s = nc.dram_tensor("bk_s", [P, 2], F32, kind="ExternalInput")
    y = nc.dram_tensor("y", [S, D], BF16, kind="ExternalOutput")

    # greedy two-engine load balancer for elementwise work (Pool/GPSIMD
    # cannot access PSUM, and every elementwise op here reads PSUM).
    ew = {"A": 0.0, "D": 0.0}

    def pick(cost_a, cost_d, force=None):
        if force is None:
            e = "A" if ew["A"] + cost_a <= ew["D"] + cost_d else "D"
        else:
            e = force
        ew[e] += cost_a if e == "A" else cost_d
        return e

    def c_act(n):  # ACT engine busy ns for n-col op
        return n * 0.833 + 185
    def c_dve(n, half=False):
        return n * (0.521 if half else 1.042) + 125

    with tile.TileContext(nc) as tc:
        with (
            tc.tile_pool(name="persist", bufs=1) as persist,
            tc.tile_pool(name="xs", bufs=8) as xs,
            tc.tile_pool(name="stp", bufs=2, space="PSUM") as stp,
            tc.tile_pool(name="ub", bufs=2, space="PSUM") as ub,
            tc.tile_pool(name="scr", bufs=2, space="PSUM") as scr,
            tc.tile_pool(name="et", bufs=12) as etp,
            tc.tile_pool(name="rz", bufs=4) as rzp,
            tc.tile_pool(name="ysb", bufs=6) as ysb,
        ):
            # ---- persistent SBUF ----
            wq_sb = persist.tile([P, ND, DC], BF16)
            wk_sb = persist.tile([P, ND, DC], BF16)
            wv_sb = persist.tile([P, ND, DC], BF16)
            bq_sb = persist.tile([P, 2], F32)
            bk_sb = persist.tile([P, 2], F32)
            expc_sb = persist.tile([P, 1], F32)
            wk_ap = wk_t.ap().rearrange("(c p) m -> p c m", p=P)
            wq_ap = wq_t.ap().rearrange("(c p) m -> p c m", p=P)
            # mt0 column-halves of Wk/Wq land first so the first projection
            # chain (k-mt0-h0 -> q-mt0-h0/h1 -> score tick 0) starts ~5us
            # earlier than with full-weight transfers
            nc.sync.dma_start(out=wk_sb[:, :, 0:P], in_=wk_ap[:, :, 0:P])
            nc.sync.dma_start(out=bk_sb, in_=bk_s.ap())
            nc.sync.dma_start(out=bq_sb, in_=bq_s.ap())
            nc.gpsimd.memset(expc_sb, float(EXPC_SHIFT))
            woc_sb = persist.tile([P, 2, D], BF16)

            qt_sb = [persist.tile([P, S], F32R, name=f"qt{t}") for t in range(2)]
            kt_sb = [persist.tile([P, S], F32R, name=f"kt{t}") for t in range(2)]
            # vaug[p, jc, h, d]; d==64 is the ones column for Z
            vaug = persist.tile([P, NJ, G, 65], FP16)
            nc.vector.memset(vaug[:, :, :, 64:65], 1.0)
            o_sb = persist.tile([P, NSC * 4, DC], BF16)
            ot_sb = [persist.tile([P, S], BF16, name=f"ot{t}") for t in range(2)]
            ident_f = persist.tile([P, P], F32)
            make_identity(nc, ident_f)
            ident = persist.tile([P, P], BF16)
            nc.vector.tensor_copy(ident, ident_f)

            # ---- work items ----
            slabs = {}
            emitted = set()

            def dma_half(key, xt, cb, hf, eng=None):
                # one 256-col s-half of an input slab: consumers of the first
                # half start after half the transfer latency (subtile deps).
                # eng selects the issuing HWDGE queue (SP default; ACT gives
                # a second parallel DMA stream during startup).
                if (key, cb) not in slabs:
                    slabs[(key, cb)] = xs.tile([P, ND, SC], BF16, tag="x", name="x")
                t = slabs[(key, cb)]
                src_ap = xt.ap().rearrange("(c p) s -> p c s", p=P)
                (eng or nc.sync).dma_start(
                    out=t[:, :, hf * HW_ : (hf + 1) * HW_],
                    in_=src_ap[
                        :, :, cb * SC + hf * HW_ : cb * SC + (hf + 1) * HW_
                    ],
                )

            def dma_slab(key, xt, cb):
                for hf in range(2):
                    dma_half(key, xt, cb, hf)
                emitted.add(("dma", key, cb))

            HW_ = SC // 2  # 256-col half for short PSUM holds

            def proj_qk(key, cb, mt, half, w_sb, b_sb, out_tiles, scale):
                ps = scr.tile([P, HW_], F32, tag="scr", name="ps")
                slab = slabs[(key, cb)]
                for dc in range(ND):
                    nc.tensor.matmul(
                        ps,
                        w_sb[:, dc, mt * P : (mt + 1) * P],
                        slab[:, dc, half * HW_ : (half + 1) * HW_],
                        start=(dc == 0),
                        stop=(dc == ND - 1),
                    )
                dst = out_tiles[mt][:, cb * SC + half * HW_ : cb * SC + (half + 1) * HW_]
                e = pick(c_act(HW_), c_dve(HW_))
                if e == "A":
                    nc.scalar.activation(
                        dst, ps, IDENT, bias=b_sb[:, mt : mt + 1], scale=scale
                    )
                else:
                    nc.vector.tensor_scalar(
                        dst, ps, scale, b_sb[:, mt : mt + 1], op0=MULT, op1=ADD
                    )
                emitted.add((key, cb, mt, half))

            def proj_v(jc):
                # j chunk jc (128 rows) -> vaug[:, jc, :, 0:64]
                cb, jq = divmod(jc, 4)
                ps = scr.tile([P, DC], F32, tag="scr", name="ps")
                slab = slabs[("v", cb)]
                for dc in range(ND):
                    nc.tensor.matmul(
                        ps,
                        slab[:, dc, jq * P : (jq + 1) * P],
                        wv_sb[:, dc, :],
                        start=(dc == 0),
                        stop=(dc == ND - 1),
                    )
                dst = vaug[:, jc, :, 0:64]
                src = ps.rearrange("p (h c) -> p h c", h=G)
                e = pick(c_act(DC), c_dve(DC))
                if e == "A":
                    nc.scalar.activation(dst, src, COPY)
                else:
                    nc.vector.tensor_copy(dst, src)
                emitted.add(("v", jc))

            def transp2(ib, mt, pr, fast=False):
                # XBAR DMA transpose (2-byte, SBUF->SBUF): no PE or ACT/DVE
                # work at all; o_sb [i, dc] bf16 -> ot_sb [dc, i].
                # fast=True (kernel tail): PE transpose + A/D copy instead --
                # the PE is idle there and XBAR DMA latency (~1.5us) would
                # sit on the critical path.
                for it in range(ib * 4 + 2 * pr, ib * 4 + 2 * pr + 2):
                    if fast:
                        tp = scr.tile([P, P], BF16, tag="scr", name="tp")
                        nc.tensor.transpose(tp, o_sb[:, it, mt * P : (mt + 1) * P], ident)
                        dst = ot_sb[mt][:, it * P : (it + 1) * P]
                        e = pick(c_act(P), c_dve(P, half=True))
                        if e == "A":
                            nc.scalar.activation(dst, tp, COPY)
                        else:
                            nc.vector.tensor_copy(dst, tp)
                    else:
                        nc.sync.dma_start(
                            out=ot_sb[mt][:, it * P : (it + 1) * P],
                            in_=o_sb[:, it, mt * P : (mt + 1) * P],
                            transpose=True,
                        )

            def oproj(it, fast=False):
                # each half flies to HBM right after its stage copy, and the
                # two copies go to different engines: halves the tail chain
                ys = ysb.tile([P, D], BF16, tag="ysb", name="ysb")
                engines = []
                for nh in range(2):
                    yp = scr.tile([P, SC], F32, tag="scr", name="yp")
                    for mt in range(2):
                        nc.tensor.matmul(
                            yp,
                            ot_sb[mt][:, it * P : (it + 1) * P],
                            woc_sb[:, mt, nh * SC : (nh + 1) * SC],
                            start=(mt == 0),
                            stop=(mt == 1),
                        )
                    dst = ys[:, nh * SC : (nh + 1) * SC]
                    if fast:
                        force = None if nh == 0 else (
                            "D" if engines[0] == "A" else "A")
                        e = pick(c_act(SC), c_dve(SC), force=force)
                    else:
                        e = pick(c_act(SC), c_dve(SC))
                    engines.append(e)
                    if e == "A":
                        nc.scalar.activation(dst, yp, COPY)
                    else:
                        nc.vector.tensor_copy(dst, yp)
                    nc.sync.dma_start(
                        out=y.ap()[it * P : (it + 1) * P, nh * SC : (nh + 1) * SC],
                        in_=dst,
                    )

            work = deque()
            light = deque()
            slow = deque()  # deferred PE work, drained 1 per 4 ticks so the
            # attention-only phase (where scores+PV under-fill the PE) stays
            # supplied all the way to the last i-block

            def drain(tick):
                if light:
                    light.popleft()()
                if work:
                    work.popleft()()
                elif slow and tick % 4 == 0:
                    slow.popleft()()

            def drain_until(key):
                while key not in emitted:
                    assert work or light or slow, f"work exhausted before {key}"
                    if light:
                        light.popleft()()
                    elif work:
                        work.popleft()()
                    else:
                        slow.popleft()()

            # ---- pipeline fill: K/Q/V chunk 0 ----
            # Two parallel DMA streams: SP carries weights + xq, ACT carries
            # xk + xv, so the k-proj -> q-proj -> tick-0 chain is fed by both
            # queues concurrently.
            dma_half("k", xk_t, 0, 0)
            proj_qk("k", 0, 0, 0, wk_sb, bk_sb, kt_sb, 1.0)
            nc.sync.dma_start(out=wq_sb[:, :, 0:P], in_=wq_ap[:, :, 0:P])
            dma_half("q", xq_t, 0, 0)
            dma_half("q", xq_t, 0, 1)
            emitted.add(("dma", "q", 0))
            proj_qk("q", 0, 0, 0, wq_sb, bq_sb, qt_sb, KPRE)
            proj_qk("q", 0, 0, 1, wq_sb, bq_sb, qt_sb, KPRE)
            dma_half("k", xk_t, 0, 1)
            emitted.add(("dma", "k", 0))
            nc.sync.dma_start(
                out=wv_sb, in_=wv_t.ap().rearrange("(c p) m -> p c m", p=P))
            dma_half("v", xv_t, 0, 0)
            dma_half("v", xv_t, 0, 1)
            emitted.add(("dma", "v", 0))
            nc.sync.dma_start(out=wk_sb[:, :, P:DC], in_=wk_ap[:, :, P:DC])
            nc.sync.dma_start(out=wq_sb[:, :, P:DC], in_=wq_ap[:, :, P:DC])

            def load_woc():
                nc.sync.dma_start(
                    out=woc_sb, in_=wo_t.ap().rearrange("(t p) n -> p t n", p=P)
                )

            work.append(lambda: proj_qk("k", 0, 0, 1, wk_sb, bk_sb, kt_sb, 1.0))
            work.append(lambda: proj_qk("k", 0, 1, 0, wk_sb, bk_sb, kt_sb, 1.0))
            work.append(lambda: proj_qk("k", 0, 1, 1, wk_sb, bk_sb, kt_sb, 1.0))
            work.append(lambda: proj_qk("q", 0, 1, 0, wq_sb, bq_sb, qt_sb, KPRE))
            work.append(lambda: proj_qk("q", 0, 1, 1, wq_sb, bq_sb, qt_sb, KPRE))
            for jc in range(4):
                work.append(lambda jc=jc: proj_v(jc))
            # K slabs lead V slabs in the DMA queue: ib0's score stream
            # consumes K chunks at nearly full DMA bandwidth
            light.append(lambda: dma_slab("k", xk_t, 1))
            light.append(lambda: dma_slab("v", xv_t, 1))
            light.append(lambda: dma_slab("k", xk_t, 2))
            light.append(lambda: dma_slab("k", xk_t, 3))
            light.append(lambda: dma_slab("v", xv_t, 2))
            light.append(lambda: dma_slab("v", xv_t, 3))
            for cb in range(1, NSC):
                for mt in range(2):
                    for half in range(2):
                        work.append(
                            lambda cb=cb, mt=mt, half=half: proj_qk(
                                "k", cb, mt, half, wk_sb, bk_sb, kt_sb, 1.0)
                        )
                for jc in range(cb * 4, cb * 4 + 4):
                    work.append(lambda jc=jc: proj_v(jc))
            light.append(load_woc)
            for cb in range(1, NSC):
                light.append(lambda cb=cb: dma_slab("q", xq_t, cb))
                for half in range(2):
                    slow.append(
                        lambda cb=cb, half=half: proj_qk(
                            "q", cb, 0, half, wq_sb, bq_sb, qt_sb, KPRE)
                    )
                    slow.append(
                        lambda cb=cb, half=half: proj_qk(
                            "q", cb, 1, half, wq_sb, bq_sb, qt_sb, KPRE)
                    )

            # ---- attention ticks: (ib, hp, J); head pair hp, 128-row j chunk J
            seq = [(ib, hp, J) for ib in range(NSC) for hp in range(2) for J in range(NJ)]
            u_tiles = {}
            et_tiles = {}

            def emit_st_exp(idx):
                ib, hp, J = seq[idx]
                if J == 0:
                    drain_until(("q", ib, hp, 0))
                    drain_until(("q", ib, hp, 1))
                if ib == 0:
                    drain_until(("k", J // 4, hp, (J % 4) // 2))
                st = stp.tile([P, 2 * SC], F32, tag="st", name="st")
                for hx in range(2):
                    nc.tensor.matmul(
                        st[:, hx * SC : (hx + 1) * SC],
                        kt_sb[hp][hx * DK : (hx + 1) * DK, J * P : (J + 1) * P],
                        qt_sb[hp][hx * DK : (hx + 1) * DK, ib * SC : (ib + 1) * SC],
                        start=True,
                        stop=True,
                        tile_position=(hx * DK, 0),
                    )
                et = etp.tile([P, 2, SC], I16, tag="et", name="et")
                et_flat = et.rearrange("p a b -> p (a b)")
                # strict A/D alternation keeps the two st pipelines decoupled
                e = pick(c_act(2 * SC), c_dve(2 * SC),
                         force="A" if idx % 2 == 0 else "D")
                if e == "A":
                    nc.scalar.activation(
                        et_flat.bitcast(FP16), st, EXP, scale=LN2_1024
                    )
                else:
                    nc.vector._custom_dve(
                        exp2_op,
                        out=et_flat,
                        in0=st,
                        in1=expc_sb,
                        s0=float(EXP_BIAS),
                        s1=EXP_MAGIC,
                        imm2=float(EXPC),
                    )
                et_tiles[idx] = et

            def emit_pv(idx):
                ib, hp, J = seq[idx]
                if J == 0:
                    for hx in range(2):
                        u_tiles[(hp, hx)] = ub.tile([P, 4, 65], F32, tag="u", name="u")
                if ib == 0 and hp == 0:
                    drain_until(("v", J))
                et = et_tiles.pop(idx).bitcast(FP16)
                for hx in range(2):
                    u = u_tiles[(hp, hx)]
                    for it in range(4):
                        nc.tensor.matmul(
                            u[:, it, :],
                            et[:, hx, it * P : (it + 1) * P],
                            vaug[:, J, 2 * hp + hx, :],
                            start=(J == 0 and it == 0),
                            stop=(J == NJ - 1 and it == 3),
                            skip_group_check=True,
                            tile_position=(0, 0),
                        )
                if J == NJ - 1:
                    finish_pair(ib, hp)

            def finish_pair(ib, hp):
                last = ib == NSC - 1
                for hx in range(2):
                    u = u_tiles.pop((hp, hx))
                    h = 2 * hp + hx
                    rz = rzp.tile([P, 4, 1], F32, tag="rz", name="rz")
                    nc.vector.reciprocal(rz, u[:, :, 64:65])
                    ew["D"] += 4 * 1.042 + 170
                    for it in range(4):
                        dst = o_sb[:, ib * 4 + it, h * DK : (h + 1) * DK]
                        if last:
                            e = pick(c_act(DK), c_dve(DK),
                                     force="A" if (it + hx) % 2 == 0 else "D")
                        else:
                            e = pick(c_act(DK), c_dve(DK))
                        if e == "A":
                            nc.scalar.activation(dst, u[:, it, 0:DK], COPY, scale=rz[:, it])
                        else:
                            nc.vector.tensor_scalar(dst, u[:, it, 0:DK], rz[:, it], None, op0=MULT)
                # o -> ot transposes are XBAR DMAs: issue as soon as this
                # head-pair's o columns are written (mt == hp)
                if not (last and hp == 1):
                    light.extend(
                        [lambda pr=pr: transp2(ib, hp, pr) for pr in range(2)]
                    )
                if hp == 1:
                    if ib < NSC - 1:
                        slow.extend(
                            [lambda it=it: oproj(it) for it in range(ib * 4, ib * 4 + 4)]
                        )
                    else:
                        # tail: PE transposes + oproj interleaved per pr pair
                        for pr in range(2):
                            work.append(lambda pr=pr: transp2(ib, 1, pr, fast=True))
                            for it in range(ib * 4 + 2 * pr, ib * 4 + 2 * pr + 2):
                                work.append(lambda it=it: oproj(it, fast=True))

            SKEW = 4  # PV lags scores/exp so its matmuls never park in the
            # PE wait queue (depth 4) blocking later scores
            for idx in range(len(seq) + SKEW):
                if idx < len(seq):
                    emit_st_exp(idx)
                if idx >= SKEW:
                    emit_pv(idx - SKEW)
                drain(idx)

            while work or light or slow:
                (light or work or slow).popleft()()

    nc.compile()
    return nc


def _get_nc():
    global _NC_CACHE
    if _NC_CACHE is None:
        _NC_CACHE = _build()
    return _NC_CACHE


def _in_maps(query, key, value, wq, wk, wv, wo, bq, bk):
    bf = ml_dtypes.bfloat16
    maps = []
    for c in range(8):
        b, g = divmod(c, 4)
        sl = slice(g * DC, (g + 1) * DC)
        maps.append(
            {
                "xq_t": np.ascontiguousarray(query[:, b, :].T).astype(bf),
                "xk_t": np.ascontiguousarray(key[:, b, :].T).astype(bf),
                "xv_t": np.ascontiguousarray(value[:, b, :].T).astype(bf),
                "wq_t": np.ascontiguousarray(wq[sl, :].T).astype(bf),
                "wk_t": np.ascontiguousarray(wk[sl, :].T).astype(bf),
                "wv_t": np.ascontiguousarray(wv[sl, :].T).astype(bf),
                "wo_t": np.ascontiguousarray(wo[:, sl].T).astype(bf),
                "bq_s": np.ascontiguousarray(
                    (bq[sl] * KPRE).astype(np.float32).reshape(2, P).T
                ),
                "bk_s": np.ascontiguousarray(bk[sl].reshape(2, P).T),
            }
        )
    return maps


def kernel(
    query, key, value, wq, bq, wk, bk, wv, bv, wo, bo, **_kw
) -> np.ndarray:
    query = np.asarray(query, np.float32)
    key = np.asarray(key, np.float32)
    value = np.asarray(value, np.float32)
    wq = np.asarray(wq, np.float32)
    wk = np.asarray(wk, np.float32)
    wv = np.asarray(wv, np.float32)
    wo = np.asarray(wo, np.float32)
    bq = np.asarray(bq, np.float32)
    bk = np.asarray(bk, np.float32)
    bv = np.asarray(bv, np.float32)
    bo = np.asarray(bo, np.float32)

    nc = _get_nc()
    res = run_bass_kernel_spmd(
        nc, _in_maps(query, key, value, wq, wk, wv, wo, bq, bk),
        core_ids=list(range(8)),
    )

    out = np.zeros((S, B, D), np.float32)
    for c in range(8):
        out[:, c // 4, :] += res.results[c]["y"].astype(np.float32)
    out += bo + wo @ bv
    return out
